# revision 3
# baseline (speedup 1.0000x reference)
"""Bahdanau additive attention kernel for Trainium2 (8 NeuronCores, SPMD).

Problem (hardcoded): B=32, Tq=4, S=2048, H=1024, 2H=2048, fp32 inputs.
  q  = query[:, -1, :]                      [B, H]
  k  = transpose(keys, (1, 0, 2))           [B, S, 2H]
  wq = q @ Wa_w.T + Wa_b                    [B, H]
  uk = k @ Ua_w.T + Ua_b                    [B, S, H]
  sc = tanh(wq[:, None, :] + uk) @ Va_w.T   [B, S]   (+ Va_b, which softmax cancels)
  w  = softmax(sc, axis=-1)                 [B, S]
  ctx = w @ k                               [B, 2H]
  returns (ctx [B,1,2H], w [B,1,S])

Sharding: data-parallel over batch. 8 cores x 4 batches each; weights
replicated; no cross-core communication.

Host-side prep is layout/dtype only (slice, transpose, cast to bf16, and
pre-swizzle into the exact SBUF tile layouts the kernel consumes); every
FLOP of the reference computation runs on device.

Per-core dataflow (all matmuls bf16 with fp32 PSUM accumulation):
  - keys are fed twice, pre-swizzled on host: kt (transposed, d on
    partitions) feeds the big uk matmul; kn (natural, s on partitions)
    feeds the context matmul.  One 2 MiB DMA per (batch, chunk) each.
  - uk tiles [h=128, s=512] accumulate in PSUM over 16 d-strips; ScalarE
    applies tanh(. + bias[h]) where bias = wq[b] + Wa_b + Ua_b.
  - scores via PE with Va columns as the 1-wide stationary operand; exp on
    ScalarE with free-dim accumulate for the softmax denominator.
  - per chunk, the score row is PE-transposed out of exp_row into columns
    (deferred by one chunk so PE never waits on Scalar/Vector), and the
    context accumulates in 4 dedicated PSUM banks across all chunks of a
    batch (weights normalized at the end).
"""

import numpy as np

B, TQ, S, H = 32, 4, 2048, 1024
D2 = 2 * H
NCORES = 8
BPC = B // NCORES  # batches per core

_CACHE = {}


def _build(s=S, h=H, d2=D2, bpc=BPC, schunk=512):
    """Build the per-core Bass module. Parameterized so a scaled-down config
    can run in CoreSim; the shipped kernel uses the defaults."""
    from contextlib import ExitStack

    import concourse.bacc as bacc
    import concourse.mybir as mybir
    import concourse.tile as tile
    from concourse.masks import make_identity

    fp32 = mybir.dt.float32
    bf16 = mybir.dt.bfloat16
    AF = mybir.ActivationFunctionType
    SD = d2 // 128        # contraction strips for uk (d on partitions)
    SM = h // 128         # h tiles (uk output partitions / Va strips)
    SJ = h // 128         # contraction strips for wq
    NCH = s // schunk     # score chunks per batch
    SPC = schunk // 128   # keys strips per chunk
    NDC = max(1, d2 // 512)   # context output chunks
    DW = min(512, d2)         # context output chunk width
    NWH = max(1, h // 512)    # wq output chunks
    WW = min(512, h)          # wq output chunk width
    NST = s // 128            # keys strips per batch

    nc = bacc.Bacc(
        "TRN2", target_bir_lowering=False, enable_partition_id=False
    )

    qt_in = nc.dram_tensor("qt", [128, SJ, bpc], bf16, kind="ExternalInput").ap()
    kn_in = nc.dram_tensor(
        "kn", [bpc * NCH, 128, SPC, d2], bf16, kind="ExternalInput"
    ).ap()
    kt_in = nc.dram_tensor(
        "kt", [bpc * NCH, 128, SD, schunk], bf16, kind="ExternalInput"
    ).ap()
    uat_in = nc.dram_tensor("uat", [128, SD, h], bf16, kind="ExternalInput").ap()
    wat_in = nc.dram_tensor("wat", [128, SJ, h], bf16, kind="ExternalInput").ap()
    vac_in = nc.dram_tensor("vac", [128, SM], bf16, kind="ExternalInput").ap()
    wabc_in = nc.dram_tensor("wabc", [128, SM], fp32, kind="ExternalInput").ap()
    uabc_in = nc.dram_tensor("uabc", [128, SM], fp32, kind="ExternalInput").ap()
    ctx_out = nc.dram_tensor("ctx", [bpc, d2], fp32, kind="ExternalOutput").ap()
    w_out = nc.dram_tensor("wts", [bpc, s], fp32, kind="ExternalOutput").ap()

    with tile.TileContext(nc) as tc:
        with ExitStack() as ctx:
            consts = ctx.enter_context(tc.tile_pool(name="consts", bufs=1))
            knp = ctx.enter_context(tc.tile_pool(name="knp", bufs=3))
            ktp = ctx.enter_context(tc.tile_pool(name="ktp", bufs=2))
            tp = ctx.enter_context(tc.tile_pool(name="tp", bufs=SM + 1))
            rows = ctx.enter_context(tc.tile_pool(name="rows", bufs=2))
            rows2 = ctx.enter_context(tc.tile_pool(name="rows2", bufs=2))
            ps_uk = ctx.enter_context(tc.tile_pool(name="ps_uk", bufs=2, space="PSUM"))
            ps_sc = ctx.enter_context(tc.tile_pool(name="ps_sc", bufs=2, space="PSUM"))
            ps_cx = ctx.enter_context(
                tc.tile_pool(name="ps_cx", bufs=NDC, space="PSUM")
            )

            # ---------------- one-time setup ----------------
            ident = consts.tile([128, 128], fp32)
            make_identity(nc, ident)

            # small vectors (gpsimd queue)
            qt = consts.tile([128, SJ, bpc], bf16)
            nc.gpsimd.dma_start(out=qt, in_=qt_in)
            vac = consts.tile([128, SM], bf16)
            nc.gpsimd.dma_start(out=vac, in_=vac_in)
            wabc = consts.tile([128, SM], fp32)
            nc.gpsimd.dma_start(out=wabc, in_=wabc_in)
            uabc = consts.tile([128, SM], fp32)
            nc.gpsimd.dma_start(out=uabc, in_=uabc_in)

            # Ua^T strips: sync queue first (they gate the first uk matmuls),
            # in 4 slices so the d=0 strips land early.
            uat = consts.tile([128, SD, h], bf16)
            sd4 = max(1, SD // 4)
            for g in range(0, SD, sd4):
                nc.sync.dma_start(
                    out=uat[:, g : g + sd4, :], in_=uat_in[:, g : g + sd4, :]
                )

            seq = [(b, c) for b in range(bpc) for c in range(NCH)]

            ktg_tiles = {}
            kn_tiles = {}

            def load_ktg(pos):
                b, c = seq[pos]
                t = ktp.tile(
                    [128, SD, schunk], bf16, tag="ktg", name=f"ktg_{b}_{c}"
                )
                nc.sync.dma_start(out=t, in_=kt_in[b * NCH + c])
                ktg_tiles[pos] = t

            def load_kn(pos):
                b, c = seq[pos]
                t = knp.tile([128, SPC, d2], bf16, tag="kn", name=f"kn_{b}_{c}")
                nc.gpsimd.dma_start(out=t, in_=kn_in[b * NCH + c])
                kn_tiles[pos] = t

            load_ktg(0)
            # Wa^T (gpsimd; only gates the wq chain)
            wat = consts.tile([128, SJ, h], bf16)
            nc.gpsimd.dma_start(out=wat, in_=wat_in)
            if len(seq) > 1:
                load_ktg(1)
            load_kn(0)
            if len(seq) > 1:
                load_kn(1)
            if len(seq) > 2:
                load_kn(2)

            # combined additive bias columns (Wa_b + Ua_b)
            comb = consts.tile([128, SM], fp32)
            nc.vector.tensor_tensor(
                out=comb, in0=wabc, in1=uabc, op=mybir.AluOpType.add
            )

            # wq = q @ Wa^T, computed as [bpc, h] with q^T strips stationary
            wq_sb = rows.tile([bpc, h], fp32, tag="wq")
            for wh in range(NWH):
                pw = ps_uk.tile([bpc, WW], fp32, tag="uk")
                for jj in range(SJ):
                    nc.tensor.matmul(
                        out=pw,
                        lhsT=qt[:, jj, :],
                        rhs=wat[:, jj, wh * WW : (wh + 1) * WW],
                        start=(jj == 0),
                        stop=(jj == SJ - 1),
                    )
                nc.vector.tensor_copy(out=wq_sb[:, wh * WW : (wh + 1) * WW], in_=pw)

            # bias_cols[:, m, b] = wq[b, 128m:128m+128].T + (Wa_b + Ua_b) cols
            bias_cols = consts.tile([128, SM, bpc], fp32)
            for m in range(SM):
                pt = ps_sc.tile([128, bpc], fp32, tag="sc")
                nc.tensor.transpose(
                    out=pt,
                    in_=wq_sb[:bpc, m * 128 : (m + 1) * 128],
                    identity=ident[:bpc, :bpc],
                )
                nc.vector.tensor_scalar_add(
                    out=bias_cols[:, m, :], in0=pt, scalar1=comb[:, m : m + 1]
                )

            # ---------------- main loop over (batch, chunk) ----------------
            state = {}

            def new_batch_state(b):
                state[b] = {
                    "exp_row": rows.tile(
                        [1, s], fp32, tag="exp_row", name=f"exp_row_{b}"
                    ),
                    "tparts": rows2.tile(
                        [1, NCH], fp32, tag="tparts", name=f"tparts_{b}"
                    ),
                    "ecols": rows2.tile(
                        [128, NST], bf16, tag="ecols", name=f"ecols_{b}"
                    ),
                    "cx": [None] * NDC,
                }

            def emit_finish(pos):
                # transpose chunk c's exp slice into columns, then accumulate
                # this chunk's context partials into the batch's PSUM banks
                b, c = seq[pos]
                st = state[b]
                pscT = ps_sc.tile([128, SPC], fp32, tag="sc")
                for g in range(SPC):
                    nc.tensor.transpose(
                        out=pscT[:, g : g + 1],
                        in_=st["exp_row"][
                            :1, c * schunk + g * 128 : c * schunk + (g + 1) * 128
                        ],
                        identity=ident[:1, :1],
                    )
                nc.vector.tensor_copy(
                    out=st["ecols"][:, c * SPC : (c + 1) * SPC], in_=pscT
                )
                for i in range(SPC):
                    for jd in range(NDC):
                        if c == 0 and i == 0:
                            st["cx"][jd] = ps_cx.tile(
                                [1, DW], fp32, tag="cx", name=f"cx_{b}_{jd}"
                            )
                        nc.tensor.matmul(
                            out=st["cx"][jd],
                            lhsT=st["ecols"][:, c * SPC + i : c * SPC + i + 1],
                            rhs=kn_tiles[pos][:, i, jd * DW : (jd + 1) * DW],
                            start=(c == 0 and i == 0),
                            stop=(c == NCH - 1 and i == SPC - 1),
                            skip_group_check=True,
                        )

            def emit_epilogue(b):
                # softmax denominator; normalize weights + context, write out
                st = state[b]
                tsum = rows2.tile([1, 1], fp32, tag="tsum")
                nc.vector.reduce_sum(
                    out=tsum, in_=st["tparts"], axis=mybir.AxisListType.X
                )
                invt = rows2.tile([1, 1], fp32, tag="invt")
                nc.vector.reciprocal(out=invt, in_=tsum)
                nc.vector.tensor_scalar_mul(
                    out=st["exp_row"], in0=st["exp_row"], scalar1=invt
                )
                nc.gpsimd.dma_start(out=w_out[b : b + 1, :], in_=st["exp_row"])
                ctx_row = rows.tile([1, d2], fp32, tag="ctx_row")
                for jd in range(NDC):
                    nc.vector.tensor_copy(
                        out=ctx_row[:, jd * DW : (jd + 1) * DW], in_=st["cx"][jd]
                    )
                nc.vector.tensor_scalar_mul(out=ctx_row, in0=ctx_row, scalar1=invt)
                nc.gpsimd.dma_start(out=ctx_out[b : b + 1, :], in_=ctx_row)

            for pos, (b, c) in enumerate(seq):
                if c == 0:
                    new_batch_state(b)
                # prefetch (ktp bufs=2 -> one ahead; knp bufs=3 -> two ahead)
                if pos + 2 < len(seq):
                    load_ktg(pos + 2) if (pos + 2) not in ktg_tiles else None
                if pos + 3 < len(seq) and (pos + 3) not in kn_tiles:
                    load_kn(pos + 3)

                # uk tiles + tanh
                ts_list = []
                for m in range(SM):
                    puk = ps_uk.tile([128, schunk], fp32, tag="uk")
                    for dd in range(SD):
                        nc.tensor.matmul(
                            out=puk,
                            lhsT=uat[:, dd, m * 128 : (m + 1) * 128],
                            rhs=ktg_tiles[pos][:, dd, :],
                            start=(dd == 0),
                            stop=(dd == SD - 1),
                        )
                    t_sb = tp.tile([128, schunk], bf16, tag="t", name=f"t_{pos}_{m}")
                    nc.scalar.activation(
                        out=t_sb,
                        in_=puk,
                        func=AF.Tanh,
                        bias=bias_cols[:, m, b : b + 1],
                        scale=1.0,
                    )
                    ts_list.append(t_sb)

                # finish the previous chunk (its exp row is ready by now)
                if pos > 0:
                    emit_finish(pos - 1)
                    pb, pc = seq[pos - 1]
                    if pc == NCH - 1:
                        emit_epilogue(pb)

                # scores for this chunk
                psc = ps_sc.tile([1, schunk], fp32, tag="sc")
                for m in range(SM):
                    nc.tensor.matmul(
                        out=psc,
                        lhsT=vac[:, m : m + 1],
                        rhs=ts_list[m],
                        start=(m == 0),
                        stop=(m == SM - 1),
                    )
                # exp row chunk (no max subtraction; scores are O(1)) and
                # the chunk's softmax partial sum
                nc.scalar.activation(
                    out=state[b]["exp_row"][:, c * schunk : (c + 1) * schunk],
                    in_=psc,
                    func=AF.Exp,
                    accum_out=state[b]["tparts"][:, c : c + 1],
                )

            emit_finish(len(seq) - 1)
            emit_epilogue(bpc - 1)

    nc.compile()
    return nc


def _prep_core_inputs(q_last, keys_bf, b0, bpc, s, h, d2, schunk):
    """Host-side layout prep for one core: slice this core's batches and
    swizzle into the exact DRAM layouts the kernel DMAs from. Layout/dtype
    only -- no arithmetic."""
    import ml_dtypes

    bf16 = ml_dtypes.bfloat16
    SD = d2 // 128
    SJ = h // 128
    NCH = s // schunk
    SPC = schunk // 128

    kn = np.empty((bpc * NCH, 128, SPC, d2), dtype=bf16)
    kt = np.empty((bpc * NCH, 128, SD, schunk), dtype=bf16)
    for b in range(bpc):
        ks = keys_bf[:, b0 + b, :]  # [s, d2] (strided view)
        # kn[b,c][p, i, x] = ks[c*schunk + i*128 + p, x]
        kn[b * NCH : (b + 1) * NCH] = ks.reshape(NCH, SPC, 128, d2).transpose(
            0, 2, 1, 3
        )
        # kt[b,c][p, dd, x] = ks[c*schunk + x, dd*128 + p]
        kt[b * NCH : (b + 1) * NCH] = ks.reshape(NCH, schunk, SD, 128).transpose(
            0, 3, 2, 1
        )

    # qt[p, j, b] = q_last[b0+b, j*128+p]
    qt = np.ascontiguousarray(
        q_last[b0 : b0 + bpc].T.reshape(SJ, 128, bpc).transpose(1, 0, 2)
    ).astype(bf16)
    return {"qt": qt, "kn": kn, "kt": kt}


def _make_in_maps(inputs):
    import ml_dtypes

    bf16 = ml_dtypes.bfloat16
    q_last = np.ascontiguousarray(
        np.asarray(inputs["query"], dtype=np.float32)[:, -1, :]
    )  # [B, H]
    keys = np.asarray(inputs["keys"], dtype=np.float32)  # [S, B, 2H]
    keys_bf = keys.astype(bf16)
    wa = np.asarray(inputs["Wa_w"], dtype=np.float32)  # [H, H]
    ua = np.asarray(inputs["Ua_w"], dtype=np.float32)  # [H, 2H]
    va = np.asarray(inputs["Va_w"], dtype=np.float32).reshape(1, H)
    wab = np.asarray(inputs["Wa_b"], dtype=np.float32).reshape(H)
    uab = np.asarray(inputs["Ua_b"], dtype=np.float32).reshape(H)

    SD = D2 // 128
    SJ = H // 128
    SM = H // 128
    # uat[p, dd, j] = Ua_w[j, dd*128+p]
    uat = np.ascontiguousarray(
        ua.T.reshape(SD, 128, H).transpose(1, 0, 2)
    ).astype(bf16)
    # wat[p, jj, ho] = Wa_w[ho, jj*128+p]
    wat = np.ascontiguousarray(
        wa.T.reshape(SJ, 128, H).transpose(1, 0, 2)
    ).astype(bf16)
    # vac[p, m] = Va_w[0, m*128+p]
    vac = np.ascontiguousarray(va.reshape(SM, 128).T).astype(bf16)
    wabc = np.ascontiguousarray(wab.reshape(SM, 128).T)
    uabc = np.ascontiguousarray(uab.reshape(SM, 128).T)

    in_maps = []
    for c in range(NCORES):
        m = _prep_core_inputs(q_last, keys_bf, c * BPC, BPC, S, H, D2, 512)
        m.update(
            {"uat": uat, "wat": wat, "vac": vac, "wabc": wabc, "uabc": uabc}
        )
        in_maps.append(m)
    return in_maps


def run(inputs, trace=False, **kwargs):
    """Run on all 8 cores; returns ((context, weights), BassKernelResults)."""
    from concourse.bass_utils import run_bass_kernel_spmd

    if "nc" not in _CACHE:
        _CACHE["nc"] = _build()
    nc = _CACHE["nc"]
    in_maps = _make_in_maps(inputs)
    res = run_bass_kernel_spmd(
        nc, in_maps, core_ids=list(range(NCORES)), trace=trace, **kwargs
    )
    context = np.empty((B, 1, D2), dtype=np.float32)
    weights = np.empty((B, 1, S), dtype=np.float32)
    for c in range(NCORES):
        b0 = c * BPC
        context[b0 : b0 + BPC, 0, :] = res.results[c]["ctx"]
        weights[b0 : b0 + BPC, 0, :] = res.results[c]["wts"]
    return (context, weights), res


def kernel(**inputs):
    out, _ = run(inputs)
    return out


# revision 6
# speedup vs baseline: 1.0433x; 1.0433x over previous
"""Bahdanau additive attention kernel for Trainium2 (8 NeuronCores, SPMD).

Problem (hardcoded): B=32, Tq=4, S=2048, H=1024, 2H=2048, fp32 inputs.
  q  = query[:, -1, :]                      [B, H]
  k  = transpose(keys, (1, 0, 2))           [B, S, 2H]
  wq = q @ Wa_w.T + Wa_b                    [B, H]
  uk = k @ Ua_w.T + Ua_b                    [B, S, H]
  sc = tanh(wq[:, None, :] + uk) @ Va_w.T   [B, S]   (+ Va_b, which softmax cancels)
  w  = softmax(sc, axis=-1)                 [B, S]
  ctx = w @ k                               [B, 2H]
  returns (ctx [B,1,2H], w [B,1,S])

Sharding: data-parallel over batch. 8 cores x 4 batches each; weights
replicated; no cross-core communication.

Host-side prep is layout/dtype only (slice, transpose, cast to bf16, and
pre-swizzle into the exact SBUF tile layouts the kernel consumes); every
FLOP of the reference computation runs on device.

Per-core dataflow (all matmuls bf16 with fp32 PSUM accumulation):
  - keys are fed twice, pre-swizzled on host: kt (transposed, d on
    partitions) feeds the big uk matmul; kn (natural, s on partitions)
    feeds the context matmul.  One 2 MiB DMA per (batch, chunk) each.
  - uk tiles [h=128, s=512] accumulate in PSUM over 16 d-strips; ScalarE
    applies tanh(. + bias[h]) where bias = wq[b] + Wa_b + Ua_b.
  - scores via PE with Va columns as the 1-wide stationary operand; exp on
    ScalarE with free-dim accumulate for the softmax denominator.
  - per chunk, the score row is PE-transposed out of exp_row into columns
    (deferred by one chunk so PE never waits on Scalar/Vector), and the
    context accumulates in 4 dedicated PSUM banks across all chunks of a
    batch (weights normalized at the end).
"""

import numpy as np

B, TQ, S, H = 32, 4, 2048, 1024
D2 = 2 * H
NCORES = 8
BPC = B // NCORES  # batches per core

_CACHE = {}


def _build(s=S, h=H, d2=D2, bpc=BPC, schunk=512):
    """Build the per-core Bass module. Parameterized so a scaled-down config
    can run in CoreSim; the shipped kernel uses the defaults."""
    from contextlib import ExitStack

    import concourse.bacc as bacc
    import concourse.mybir as mybir
    import concourse.tile as tile
    from concourse.masks import make_identity

    fp32 = mybir.dt.float32
    bf16 = mybir.dt.bfloat16
    AF = mybir.ActivationFunctionType
    SD = d2 // 128        # contraction strips for uk (d on partitions)
    SM = h // 128         # h tiles (uk output partitions / Va strips)
    SJ = h // 128         # contraction strips for wq
    NCH = s // schunk     # score chunks per batch
    SPC = schunk // 128   # keys strips per chunk
    NDC = max(1, d2 // 512)   # context output chunks
    DW = min(512, d2)         # context output chunk width
    NWH = max(1, h // 512)    # wq output chunks
    WW = min(512, h)          # wq output chunk width
    NST = s // 128            # keys strips per batch

    nc = bacc.Bacc(
        "TRN2", target_bir_lowering=False, enable_partition_id=False
    )

    qt_in = nc.dram_tensor("qt", [128, SJ, bpc], bf16, kind="ExternalInput").ap()
    kn_in = nc.dram_tensor(
        "kn", [bpc * NCH, 128, SPC, d2], bf16, kind="ExternalInput"
    ).ap()
    kt_in = nc.dram_tensor(
        "kt", [bpc * NCH, 128, SD, schunk], bf16, kind="ExternalInput"
    ).ap()
    uat_in = nc.dram_tensor("uat", [128, SD, h], bf16, kind="ExternalInput").ap()
    wat_in = nc.dram_tensor("wat", [128, SJ, h], bf16, kind="ExternalInput").ap()
    vac_in = nc.dram_tensor("vac", [128, SM], bf16, kind="ExternalInput").ap()
    wabc_in = nc.dram_tensor("wabc", [128, SM], fp32, kind="ExternalInput").ap()
    uabc_in = nc.dram_tensor("uabc", [128, SM], fp32, kind="ExternalInput").ap()
    ctx_out = nc.dram_tensor("ctx", [bpc, d2], fp32, kind="ExternalOutput").ap()
    w_out = nc.dram_tensor("wts", [bpc, s], fp32, kind="ExternalOutput").ap()

    with tile.TileContext(nc) as tc:
        with ExitStack() as ctx:
            consts = ctx.enter_context(tc.tile_pool(name="consts", bufs=1))
            knp = ctx.enter_context(tc.tile_pool(name="knp", bufs=3))
            ktp = ctx.enter_context(tc.tile_pool(name="ktp", bufs=2))
            tp = ctx.enter_context(tc.tile_pool(name="tp", bufs=SM + 1))
            rows = ctx.enter_context(tc.tile_pool(name="rows", bufs=2))
            rows2 = ctx.enter_context(tc.tile_pool(name="rows2", bufs=2))
            ps_uk = ctx.enter_context(tc.tile_pool(name="ps_uk", bufs=2, space="PSUM"))
            ps_sc = ctx.enter_context(tc.tile_pool(name="ps_sc", bufs=2, space="PSUM"))
            ps_cx = ctx.enter_context(
                tc.tile_pool(name="ps_cx", bufs=NDC, space="PSUM")
            )

            # ---------------- one-time setup ----------------
            ident = consts.tile([128, 128], fp32)
            make_identity(nc, ident)

            # small vectors first (gpsimd queue): qt/wat gate the wq chain
            qt = consts.tile([128, SJ, bpc], bf16)
            nc.gpsimd.dma_start(out=qt, in_=qt_in)
            # Wa^T (gpsimd; only gates the wq chain)
            wat = consts.tile([128, SJ, h], bf16)
            nc.gpsimd.dma_start(out=wat, in_=wat_in)
            vac = consts.tile([128, SM], bf16)
            nc.gpsimd.dma_start(out=vac, in_=vac_in)
            wabc = consts.tile([128, SM], fp32)
            nc.gpsimd.dma_start(out=wabc, in_=wabc_in)
            uabc = consts.tile([128, SM], fp32)
            nc.gpsimd.dma_start(out=uabc, in_=uabc_in)

            seq = [(b, c) for b in range(bpc) for c in range(NCH)]

            ktg_tiles = {}
            kn_tiles = {}

            def load_ktg(pos):
                b, c = seq[pos]
                t = ktp.tile(
                    [128, SD, schunk], bf16, tag="ktg", name=f"ktg_{b}_{c}"
                )
                nc.sync.dma_start(out=t, in_=kt_in[b * NCH + c])
                ktg_tiles[pos] = t

            def load_kn(pos, queue):
                b, c = seq[pos]
                t = knp.tile([128, SPC, d2], bf16, tag="kn", name=f"kn_{b}_{c}")
                queue.dma_start(out=t, in_=kn_in[b * NCH + c])
                kn_tiles[pos] = t

            # Startup-critical loads on sync, interleaved at d-strip-group
            # granularity so chunk 0's d-outer matmuls can trickle behind the
            # DMA front: [uat d0-1, d2-3, ktg0 d0-3, uat d4-5, ...].
            uat = consts.tile([128, SD, h], bf16)
            ktg0 = ktp.tile([128, SD, schunk], bf16, tag="ktg", name="ktg_0_0")
            ktg_tiles[0] = ktg0
            sd2 = max(1, SD // 8)
            sd4 = max(1, SD // 4)
            ug = [(g, min(g + sd2, SD)) for g in range(0, SD, sd2)]
            kq = [(g, min(g + sd4, SD)) for g in range(0, SD, sd4)]
            while ug or kq:
                for _ in range(2):
                    if ug:
                        a, b_ = ug.pop(0)
                        nc.sync.dma_start(
                            out=uat[:, a:b_, :], in_=uat_in[:, a:b_, :]
                        )
                if kq:
                    a, b_ = kq.pop(0)
                    nc.sync.dma_start(
                        out=ktg0[:, a:b_, :], in_=kt_in[0][:, a:b_, :]
                    )
            if len(seq) > 1:
                load_ktg(1)
            # First kn chunks go on sync BEHIND the critical path (they are
            # not needed until the deferred context of chunk 0/1/2), so they
            # don't steal HBM bandwidth from uat/ktg0.
            for p in range(min(3, len(seq))):
                load_kn(p, nc.sync)

            # combined additive bias columns (Wa_b + Ua_b)
            comb = consts.tile([128, SM], fp32)
            nc.vector.tensor_tensor(
                out=comb, in0=wabc, in1=uabc, op=mybir.AluOpType.add
            )

            # wq = q @ Wa^T, computed as [bpc, h] with q^T strips stationary
            wq_sb = rows.tile([bpc, h], fp32, tag="wq")
            for wh in range(NWH):
                pw = ps_uk.tile([bpc, WW], fp32, tag="uk")
                for jj in range(SJ):
                    nc.tensor.matmul(
                        out=pw,
                        lhsT=qt[:, jj, :],
                        rhs=wat[:, jj, wh * WW : (wh + 1) * WW],
                        start=(jj == 0),
                        stop=(jj == SJ - 1),
                    )
                nc.vector.tensor_copy(out=wq_sb[:, wh * WW : (wh + 1) * WW], in_=pw)

            # bias_cols[:, m, b] = wq[b, 128m:128m+128].T + (Wa_b + Ua_b) cols
            bias_cols = consts.tile([128, SM, bpc], fp32)
            for m in range(SM):
                pt = ps_sc.tile([128, bpc], fp32, tag="sc")
                nc.tensor.transpose(
                    out=pt,
                    in_=wq_sb[:bpc, m * 128 : (m + 1) * 128],
                    identity=ident[:bpc, :bpc],
                )
                nc.vector.tensor_scalar_add(
                    out=bias_cols[:, m, :], in0=pt, scalar1=comb[:, m : m + 1]
                )

            # ---------------- main loop over (batch, chunk) ----------------
            state = {}

            def new_batch_state(b):
                state[b] = {
                    "exp_row": rows.tile(
                        [1, s], fp32, tag="exp_row", name=f"exp_row_{b}"
                    ),
                    "tparts": rows2.tile(
                        [1, NCH], fp32, tag="tparts", name=f"tparts_{b}"
                    ),
                    "ecols": rows2.tile(
                        [128, NST], bf16, tag="ecols", name=f"ecols_{b}"
                    ),
                    "cx": [None] * NDC,
                }

            def emit_finish(pos):
                # transpose chunk c's exp slice into columns, then accumulate
                # this chunk's context partials into the batch's PSUM banks;
                # on the last chunk, scale each finished bank out to SBUF
                b, c = seq[pos]
                st = state[b]
                pscT = ps_sc.tile([128, SPC], fp32, tag="sc")
                for g in range(SPC):
                    nc.tensor.transpose(
                        out=pscT[:, g : g + 1],
                        in_=st["exp_row"][
                            :1, c * schunk + g * 128 : c * schunk + (g + 1) * 128
                        ],
                        identity=ident[:1, :1],
                    )
                nc.vector.tensor_copy(
                    out=st["ecols"][:, c * SPC : (c + 1) * SPC], in_=pscT
                )
                for jd in range(NDC):
                    for i in range(SPC):
                        if c == 0 and i == 0:
                            st["cx"][jd] = ps_cx.tile(
                                [1, DW], fp32, tag="cx", name=f"cx_{b}_{jd}"
                            )
                        nc.tensor.matmul(
                            out=st["cx"][jd],
                            lhsT=st["ecols"][:, c * SPC + i : c * SPC + i + 1],
                            rhs=kn_tiles[pos][:, i, jd * DW : (jd + 1) * DW],
                            start=(c == 0 and i == 0),
                            stop=(c == NCH - 1 and i == SPC - 1),
                            skip_group_check=True,
                        )
                    if c == NCH - 1:
                        nc.vector.tensor_scalar_mul(
                            out=st["ctx_row"][:, jd * DW : (jd + 1) * DW],
                            in0=st["cx"][jd],
                            scalar1=st["invt"],
                        )
                if c == NCH - 1:
                    nc.gpsimd.dma_start(
                        out=ctx_out[b : b + 1, :], in_=st["ctx_row"]
                    )

            for pos, (b, c) in enumerate(seq):
                if c == 0:
                    new_batch_state(b)
                # prefetch (ktp bufs=2 -> one ahead; knp bufs=3 -> two ahead)
                if pos + 2 < len(seq) and (pos + 2) not in ktg_tiles:
                    load_ktg(pos + 2)
                if pos + 3 < len(seq) and (pos + 3) not in kn_tiles:
                    load_kn(pos + 3, nc.gpsimd)

                # uk tiles + tanh.  Chunk 0 runs d-outer with all 8 m-psums
                # live at once (borrowing every PSUM bank) so the PE can
                # consume uat/ktg0 d-strips as the startup DMAs land instead
                # of stalling for the full Ua^T before finishing any m-tile.
                ts_list = []
                if pos == 0:
                    pps = []
                    for m in range(SM):
                        pool = [ps_uk, ps_sc, ps_cx, ps_cx][m * 4 // SM]
                        tag = ["uk", "sc", "cx", "cx"][m * 4 // SM]
                        pps.append(
                            pool.tile(
                                [128, schunk], fp32, tag=tag, name=f"puk0_{m}"
                            )
                        )
                    for dd in range(SD):
                        for m in range(SM):
                            nc.tensor.matmul(
                                out=pps[m],
                                lhsT=uat[:, dd, m * 128 : (m + 1) * 128],
                                rhs=ktg_tiles[pos][:, dd, :],
                                start=(dd == 0),
                                stop=(dd == SD - 1),
                                skip_group_check=True,
                            )
                    for m in range(SM):
                        t_sb = tp.tile(
                            [128, schunk], bf16, tag="t", name=f"t_{pos}_{m}"
                        )
                        nc.scalar.activation(
                            out=t_sb,
                            in_=pps[m],
                            func=AF.Tanh,
                            bias=bias_cols[:, m, b : b + 1],
                            scale=1.0,
                        )
                        ts_list.append(t_sb)
                else:
                    for m in range(SM):
                        puk = ps_uk.tile([128, schunk], fp32, tag="uk")
                        for dd in range(SD):
                            nc.tensor.matmul(
                                out=puk,
                                lhsT=uat[:, dd, m * 128 : (m + 1) * 128],
                                rhs=ktg_tiles[pos][:, dd, :],
                                start=(dd == 0),
                                stop=(dd == SD - 1),
                            )
                        t_sb = tp.tile(
                            [128, schunk], bf16, tag="t", name=f"t_{pos}_{m}"
                        )
                        nc.scalar.activation(
                            out=t_sb,
                            in_=puk,
                            func=AF.Tanh,
                            bias=bias_cols[:, m, b : b + 1],
                            scale=1.0,
                        )
                        ts_list.append(t_sb)

                # finish the previous chunk (its exp row is ready by now)
                if pos > 0:
                    emit_finish(pos - 1)

                # scores for this chunk
                psc = ps_sc.tile([1, schunk], fp32, tag="sc")
                for m in range(SM):
                    nc.tensor.matmul(
                        out=psc,
                        lhsT=vac[:, m : m + 1],
                        rhs=ts_list[m],
                        start=(m == 0),
                        stop=(m == SM - 1),
                    )
                # exp row chunk (no max subtraction; scores are O(1)) and
                # the chunk's softmax partial sum
                st = state[b]
                nc.scalar.activation(
                    out=st["exp_row"][:, c * schunk : (c + 1) * schunk],
                    in_=psc,
                    func=AF.Exp,
                    accum_out=st["tparts"][:, c : c + 1],
                )
                if c == NCH - 1:
                    # softmax denominator + normalized weights can go out now;
                    # the raw exp_row stays untouched for the deferred context
                    tsum = rows2.tile([1, 1], fp32, tag="tsum", name=f"tsum_{b}")
                    nc.vector.reduce_sum(
                        out=tsum, in_=st["tparts"], axis=mybir.AxisListType.X
                    )
                    invt = rows2.tile([1, 1], fp32, tag="invt", name=f"invt_{b}")
                    nc.vector.reciprocal(out=invt, in_=tsum)
                    st["invt"] = invt
                    w_norm = rows.tile([1, s], fp32, tag="w_norm", name=f"wn_{b}")
                    nc.vector.tensor_scalar_mul(
                        out=w_norm, in0=st["exp_row"], scalar1=invt
                    )
                    nc.gpsimd.dma_start(out=w_out[b : b + 1, :], in_=w_norm)
                    st["ctx_row"] = rows.tile(
                        [1, d2], fp32, tag="ctx_row", name=f"cr_{b}"
                    )

            emit_finish(len(seq) - 1)

    nc.compile()
    return nc


def _prep_core_inputs(q_last, keys_bf, b0, bpc, s, h, d2, schunk):
    """Host-side layout prep for one core: slice this core's batches and
    swizzle into the exact DRAM layouts the kernel DMAs from. Layout/dtype
    only -- no arithmetic."""
    import ml_dtypes

    bf16 = ml_dtypes.bfloat16
    SD = d2 // 128
    SJ = h // 128
    NCH = s // schunk
    SPC = schunk // 128

    kn = np.empty((bpc * NCH, 128, SPC, d2), dtype=bf16)
    kt = np.empty((bpc * NCH, 128, SD, schunk), dtype=bf16)
    for b in range(bpc):
        ks = keys_bf[:, b0 + b, :]  # [s, d2] (strided view)
        # kn[b,c][p, i, x] = ks[c*schunk + i*128 + p, x]
        kn[b * NCH : (b + 1) * NCH] = ks.reshape(NCH, SPC, 128, d2).transpose(
            0, 2, 1, 3
        )
        # kt[b,c][p, dd, x] = ks[c*schunk + x, dd*128 + p]
        kt[b * NCH : (b + 1) * NCH] = ks.reshape(NCH, schunk, SD, 128).transpose(
            0, 3, 2, 1
        )

    # qt[p, j, b] = q_last[b0+b, j*128+p]
    qt = np.ascontiguousarray(
        q_last[b0 : b0 + bpc].T.reshape(SJ, 128, bpc).transpose(1, 0, 2)
    ).astype(bf16)
    return {"qt": qt, "kn": kn, "kt": kt}


def _make_in_maps(inputs):
    import ml_dtypes

    bf16 = ml_dtypes.bfloat16
    q_last = np.ascontiguousarray(
        np.asarray(inputs["query"], dtype=np.float32)[:, -1, :]
    )  # [B, H]
    keys = np.asarray(inputs["keys"], dtype=np.float32)  # [S, B, 2H]
    keys_bf = keys.astype(bf16)
    wa = np.asarray(inputs["Wa_w"], dtype=np.float32)  # [H, H]
    ua = np.asarray(inputs["Ua_w"], dtype=np.float32)  # [H, 2H]
    va = np.asarray(inputs["Va_w"], dtype=np.float32).reshape(1, H)
    wab = np.asarray(inputs["Wa_b"], dtype=np.float32).reshape(H)
    uab = np.asarray(inputs["Ua_b"], dtype=np.float32).reshape(H)

    SD = D2 // 128
    SJ = H // 128
    SM = H // 128
    # uat[p, dd, j] = Ua_w[j, dd*128+p]
    uat = np.ascontiguousarray(
        ua.T.reshape(SD, 128, H).transpose(1, 0, 2)
    ).astype(bf16)
    # wat[p, jj, ho] = Wa_w[ho, jj*128+p]
    wat = np.ascontiguousarray(
        wa.T.reshape(SJ, 128, H).transpose(1, 0, 2)
    ).astype(bf16)
    # vac[p, m] = Va_w[0, m*128+p]
    vac = np.ascontiguousarray(va.reshape(SM, 128).T).astype(bf16)
    wabc = np.ascontiguousarray(wab.reshape(SM, 128).T)
    uabc = np.ascontiguousarray(uab.reshape(SM, 128).T)

    in_maps = []
    for c in range(NCORES):
        m = _prep_core_inputs(q_last, keys_bf, c * BPC, BPC, S, H, D2, 512)
        m.update(
            {"uat": uat, "wat": wat, "vac": vac, "wabc": wabc, "uabc": uabc}
        )
        in_maps.append(m)
    return in_maps


def run(inputs, trace=False, **kwargs):
    """Run on all 8 cores; returns ((context, weights), BassKernelResults)."""
    from concourse.bass_utils import run_bass_kernel_spmd

    if "nc" not in _CACHE:
        _CACHE["nc"] = _build()
    nc = _CACHE["nc"]
    in_maps = _make_in_maps(inputs)
    res = run_bass_kernel_spmd(
        nc, in_maps, core_ids=list(range(NCORES)), trace=trace, **kwargs
    )
    context = np.empty((B, 1, D2), dtype=np.float32)
    weights = np.empty((B, 1, S), dtype=np.float32)
    for c in range(NCORES):
        b0 = c * BPC
        context[b0 : b0 + BPC, 0, :] = res.results[c]["ctx"]
        weights[b0 : b0 + BPC, 0, :] = res.results[c]["wts"]
    return (context, weights), res


def kernel(**inputs):
    out, _ = run(inputs)
    return out


# revision 9
# speedup vs baseline: 1.0506x; 1.0070x over previous
"""Bahdanau additive attention kernel for Trainium2 (8 NeuronCores, SPMD).

Problem (hardcoded): B=32, Tq=4, S=2048, H=1024, 2H=2048, fp32 inputs.
  q  = query[:, -1, :]                      [B, H]
  k  = transpose(keys, (1, 0, 2))           [B, S, 2H]
  wq = q @ Wa_w.T + Wa_b                    [B, H]
  uk = k @ Ua_w.T + Ua_b                    [B, S, H]
  sc = tanh(wq[:, None, :] + uk) @ Va_w.T   [B, S]   (+ Va_b, which softmax cancels)
  w  = softmax(sc, axis=-1)                 [B, S]
  ctx = w @ k                               [B, 2H]
  returns (ctx [B,1,2H], w [B,1,S])

Sharding: data-parallel over batch. 8 cores x 4 batches each; weights
replicated; no cross-core communication.

Host-side prep is layout/dtype only (slice, transpose, cast to bf16, and
pre-swizzle into the exact SBUF tile layouts the kernel consumes); every
FLOP of the reference computation runs on device.

Per-core dataflow (all matmuls bf16 with fp32 PSUM accumulation):
  - keys are fed twice, pre-swizzled on host: kt (transposed, d on
    partitions) feeds the big uk matmul; kn (natural, s on partitions)
    feeds the context matmul.  One 2 MiB DMA per (batch, chunk) each.
  - uk tiles [h=128, s=512] accumulate in PSUM over 16 d-strips; ScalarE
    applies tanh(. + bias[h]) where bias = wq[b] + Wa_b + Ua_b.
  - scores via PE with Va columns as the 1-wide stationary operand; exp on
    ScalarE with free-dim accumulate for the softmax denominator.
  - per chunk, the score row is PE-transposed out of exp_row into columns
    (deferred by one chunk so PE never waits on Scalar/Vector), and the
    context accumulates in 4 dedicated PSUM banks across all chunks of a
    batch (weights normalized at the end).
"""

import numpy as np

B, TQ, S, H = 32, 4, 2048, 1024
D2 = 2 * H
NCORES = 8
BPC = B // NCORES  # batches per core

_CACHE = {}


def _build(s=S, h=H, d2=D2, bpc=BPC, schunk=512):
    """Build the per-core Bass module. Parameterized so a scaled-down config
    can run in CoreSim; the shipped kernel uses the defaults."""
    from contextlib import ExitStack

    import concourse.bacc as bacc
    import concourse.mybir as mybir
    import concourse.tile as tile
    from concourse.masks import make_identity

    fp32 = mybir.dt.float32
    bf16 = mybir.dt.bfloat16
    AF = mybir.ActivationFunctionType
    SD = d2 // 128        # contraction strips for uk (d on partitions)
    SM = h // 128         # h tiles (uk output partitions / Va strips)
    SJ = h // 128         # contraction strips for wq
    NCH = s // schunk     # score chunks per batch
    SPC = schunk // 128   # keys strips per chunk
    NDC = max(1, d2 // 512)   # context output chunks
    DW = min(512, d2)         # context output chunk width
    NWH = max(1, h // 512)    # wq output chunks
    WW = min(512, h)          # wq output chunk width
    NST = s // 128            # keys strips per batch

    nc = bacc.Bacc(
        "TRN2", target_bir_lowering=False, enable_partition_id=False
    )

    qt_in = nc.dram_tensor("qt", [128, SJ, bpc], bf16, kind="ExternalInput").ap()
    kn_in = nc.dram_tensor(
        "kn", [bpc * NCH, 128, SPC, d2], bf16, kind="ExternalInput"
    ).ap()
    kt_in = nc.dram_tensor(
        "kt", [bpc * NCH, 128, SD, schunk], bf16, kind="ExternalInput"
    ).ap()
    uat_in = nc.dram_tensor("uat", [128, SD, h], bf16, kind="ExternalInput").ap()
    wat_in = nc.dram_tensor("wat", [128, SJ, h], bf16, kind="ExternalInput").ap()
    vac_in = nc.dram_tensor("vac", [128, SM], bf16, kind="ExternalInput").ap()
    wabc_in = nc.dram_tensor("wabc", [128, SM], fp32, kind="ExternalInput").ap()
    uabc_in = nc.dram_tensor("uabc", [128, SM], fp32, kind="ExternalInput").ap()
    ctx_out = nc.dram_tensor("ctx", [bpc, d2], fp32, kind="ExternalOutput").ap()
    w_out = nc.dram_tensor("wts", [bpc, s], fp32, kind="ExternalOutput").ap()

    with tile.TileContext(nc) as tc:
        with ExitStack() as ctx:
            consts = ctx.enter_context(tc.tile_pool(name="consts", bufs=1))
            knp = ctx.enter_context(tc.tile_pool(name="knp", bufs=3))
            ktp = ctx.enter_context(tc.tile_pool(name="ktp", bufs=2))
            tp = ctx.enter_context(tc.tile_pool(name="tp", bufs=SM + 1))
            rows = ctx.enter_context(tc.tile_pool(name="rows", bufs=2))
            rows2 = ctx.enter_context(tc.tile_pool(name="rows2", bufs=2))
            ps_uk = ctx.enter_context(tc.tile_pool(name="ps_uk", bufs=2, space="PSUM"))
            ps_sc = ctx.enter_context(tc.tile_pool(name="ps_sc", bufs=2, space="PSUM"))
            ps_cx = ctx.enter_context(
                tc.tile_pool(name="ps_cx", bufs=NDC, space="PSUM")
            )

            # ---------------- one-time setup ----------------
            ident = consts.tile([128, 128], fp32)
            make_identity(nc, ident)

            # small vectors first (gpsimd queue): qt/wat gate the wq chain
            qt = consts.tile([128, SJ, bpc], bf16)
            nc.gpsimd.dma_start(out=qt, in_=qt_in)
            # Wa^T (gpsimd; only gates the wq chain)
            wat = consts.tile([128, SJ, h], bf16)
            nc.gpsimd.dma_start(out=wat, in_=wat_in)
            vac = consts.tile([128, SM], bf16)
            nc.gpsimd.dma_start(out=vac, in_=vac_in)
            wabc = consts.tile([128, SM], fp32)
            nc.gpsimd.dma_start(out=wabc, in_=wabc_in)
            uabc = consts.tile([128, SM], fp32)
            nc.gpsimd.dma_start(out=uabc, in_=uabc_in)

            seq = [(b, c) for b in range(bpc) for c in range(NCH)]

            ktg_tiles = {}
            kn_tiles = {}

            def load_ktg(pos):
                b, c = seq[pos]
                t = ktp.tile(
                    [128, SD, schunk], bf16, tag="ktg", name=f"ktg_{b}_{c}"
                )
                nc.sync.dma_start(out=t, in_=kt_in[b * NCH + c])
                ktg_tiles[pos] = t

            def load_kn(pos, queue):
                b, c = seq[pos]
                t = knp.tile([128, SPC, d2], bf16, tag="kn", name=f"kn_{b}_{c}")
                queue.dma_start(out=t, in_=kn_in[b * NCH + c])
                kn_tiles[pos] = t

            # Startup-critical loads on sync, interleaved at d-strip-group
            # granularity so chunk 0's d-outer matmuls can trickle behind the
            # DMA front: [uat d0-1, d2-3, ktg0 d0-3, uat d4-5, ...].
            uat = consts.tile([128, SD, h], bf16)
            ktg0 = ktp.tile([128, SD, schunk], bf16, tag="ktg", name="ktg_0_0")
            ktg_tiles[0] = ktg0
            sd2 = max(1, SD // 8)
            sd4 = max(1, SD // 4)
            ug = [(g, min(g + sd2, SD)) for g in range(0, SD, sd2)]
            kq = [(g, min(g + sd4, SD)) for g in range(0, SD, sd4)]
            while ug or kq:
                if ug:
                    a, b_ = ug.pop(0)
                    nc.sync.dma_start(
                        out=uat[:, a:b_, :], in_=uat_in[:, a:b_, :]
                    )
                if kq:
                    a, b_ = kq.pop(0)
                    nc.sync.dma_start(
                        out=ktg0[:, a:b_, :], in_=kt_in[0][:, a:b_, :]
                    )
                if ug:
                    a, b_ = ug.pop(0)
                    nc.sync.dma_start(
                        out=uat[:, a:b_, :], in_=uat_in[:, a:b_, :]
                    )
            if len(seq) > 1:
                load_ktg(1)
            # First kn chunks go on sync BEHIND the critical path (they are
            # not needed until the deferred context of chunk 0/1/2), so they
            # don't steal HBM bandwidth from uat/ktg0.
            for p in range(min(3, len(seq))):
                load_kn(p, nc.sync)

            # combined additive bias columns (Wa_b + Ua_b)
            comb = consts.tile([128, SM], fp32)
            nc.vector.tensor_tensor(
                out=comb, in0=wabc, in1=uabc, op=mybir.AluOpType.add
            )

            # wq = q @ Wa^T, computed as [bpc, h] with q^T strips stationary
            wq_sb = rows.tile([bpc, h], fp32, tag="wq")
            for wh in range(NWH):
                pw = ps_uk.tile([bpc, WW], fp32, tag="uk")
                for jj in range(SJ):
                    nc.tensor.matmul(
                        out=pw,
                        lhsT=qt[:, jj, :],
                        rhs=wat[:, jj, wh * WW : (wh + 1) * WW],
                        start=(jj == 0),
                        stop=(jj == SJ - 1),
                    )
                nc.vector.tensor_copy(out=wq_sb[:, wh * WW : (wh + 1) * WW], in_=pw)

            # bias_cols[:, m, b] = wq[b, 128m:128m+128].T + (Wa_b + Ua_b) cols
            bias_cols = consts.tile([128, SM, bpc], fp32)
            for m in range(SM):
                pt = ps_sc.tile([128, bpc], fp32, tag="sc")
                nc.tensor.transpose(
                    out=pt,
                    in_=wq_sb[:bpc, m * 128 : (m + 1) * 128],
                    identity=ident[:bpc, :bpc],
                )
                nc.vector.tensor_scalar_add(
                    out=bias_cols[:, m, :], in0=pt, scalar1=comb[:, m : m + 1]
                )

            # ---------------- main loop over (batch, chunk) ----------------
            state = {}

            def new_batch_state(b):
                state[b] = {
                    "exp_row": rows.tile(
                        [1, s], fp32, tag="exp_row", name=f"exp_row_{b}"
                    ),
                    "tparts": rows2.tile(
                        [1, NCH], fp32, tag="tparts", name=f"tparts_{b}"
                    ),
                    "ecols": rows2.tile(
                        [128, NST], bf16, tag="ecols", name=f"ecols_{b}"
                    ),
                    "cx": [None] * NDC,
                }

            def emit_finish(pos):
                # transpose chunk c's exp slice into columns, then accumulate
                # this chunk's context partials into the batch's PSUM banks;
                # on the last chunk, scale each finished bank out to SBUF
                b, c = seq[pos]
                st = state[b]
                pscT = ps_sc.tile([128, SPC], fp32, tag="sc")
                for g in range(SPC):
                    nc.tensor.transpose(
                        out=pscT[:, g : g + 1],
                        in_=st["exp_row"][
                            :1, c * schunk + g * 128 : c * schunk + (g + 1) * 128
                        ],
                        identity=ident[:1, :1],
                    )
                nc.vector.tensor_copy(
                    out=st["ecols"][:, c * SPC : (c + 1) * SPC], in_=pscT
                )
                for jd in range(NDC):
                    for i in range(SPC):
                        if c == 0 and i == 0:
                            st["cx"][jd] = ps_cx.tile(
                                [1, DW], fp32, tag="cx", name=f"cx_{b}_{jd}"
                            )
                        nc.tensor.matmul(
                            out=st["cx"][jd],
                            lhsT=st["ecols"][:, c * SPC + i : c * SPC + i + 1],
                            rhs=kn_tiles[pos][:, i, jd * DW : (jd + 1) * DW],
                            start=(c == 0 and i == 0),
                            stop=(c == NCH - 1 and i == SPC - 1),
                            skip_group_check=True,
                        )
                    if c == NCH - 1:
                        # scale finished banks out, split across DVE and ACT
                        # so the tail chain isn't serialized on one engine
                        if jd % 2 == 0:
                            nc.vector.tensor_scalar_mul(
                                out=st["ctx_row"][:, jd * DW : (jd + 1) * DW],
                                in0=st["cx"][jd],
                                scalar1=st["invt"],
                            )
                        else:
                            nc.scalar.activation(
                                out=st["ctx_row"][:, jd * DW : (jd + 1) * DW],
                                in_=st["cx"][jd],
                                func=AF.Copy,
                                scale=st["invt"],
                            )
                if c == NCH - 1:
                    q = nc.sync if b == bpc - 1 else nc.gpsimd
                    q.dma_start(out=ctx_out[b : b + 1, :], in_=st["ctx_row"])

            for pos, (b, c) in enumerate(seq):
                if c == 0:
                    new_batch_state(b)
                # prefetch (ktp bufs=2 -> one ahead; knp bufs=3 -> two ahead)
                if pos + 2 < len(seq) and (pos + 2) not in ktg_tiles:
                    load_ktg(pos + 2)
                if pos + 3 < len(seq) and (pos + 3) not in kn_tiles:
                    load_kn(pos + 3, nc.gpsimd)

                # uk tiles + tanh.  Chunk 0 runs d-outer with all 8 m-psums
                # live at once (borrowing every PSUM bank) so the PE can
                # consume uat/ktg0 d-strips as the startup DMAs land instead
                # of stalling for the full Ua^T before finishing any m-tile.
                ts_list = []
                if pos == 0:
                    pps = []
                    for m in range(SM):
                        pool = [ps_uk, ps_sc, ps_cx, ps_cx][m * 4 // SM]
                        tag = ["uk", "sc", "cx", "cx"][m * 4 // SM]
                        pps.append(
                            pool.tile(
                                [128, schunk], fp32, tag=tag, name=f"puk0_{m}"
                            )
                        )
                    for dd in range(SD):
                        for m in range(SM):
                            nc.tensor.matmul(
                                out=pps[m],
                                lhsT=uat[:, dd, m * 128 : (m + 1) * 128],
                                rhs=ktg_tiles[pos][:, dd, :],
                                start=(dd == 0),
                                stop=(dd == SD - 1),
                                skip_group_check=True,
                            )
                    for m in range(SM):
                        t_sb = tp.tile(
                            [128, schunk], bf16, tag="t", name=f"t_{pos}_{m}"
                        )
                        nc.scalar.activation(
                            out=t_sb,
                            in_=pps[m],
                            func=AF.Tanh,
                            bias=bias_cols[:, m, b : b + 1],
                            scale=1.0,
                        )
                        ts_list.append(t_sb)
                else:
                    for m in range(SM):
                        puk = ps_uk.tile([128, schunk], fp32, tag="uk")
                        for dd in range(SD):
                            nc.tensor.matmul(
                                out=puk,
                                lhsT=uat[:, dd, m * 128 : (m + 1) * 128],
                                rhs=ktg_tiles[pos][:, dd, :],
                                start=(dd == 0),
                                stop=(dd == SD - 1),
                            )
                        t_sb = tp.tile(
                            [128, schunk], bf16, tag="t", name=f"t_{pos}_{m}"
                        )
                        nc.scalar.activation(
                            out=t_sb,
                            in_=puk,
                            func=AF.Tanh,
                            bias=bias_cols[:, m, b : b + 1],
                            scale=1.0,
                        )
                        ts_list.append(t_sb)

                # finish the previous chunk (its exp row is ready by now)
                if pos > 0:
                    emit_finish(pos - 1)

                # scores for this chunk
                psc = ps_sc.tile([1, schunk], fp32, tag="sc")
                for m in range(SM):
                    nc.tensor.matmul(
                        out=psc,
                        lhsT=vac[:, m : m + 1],
                        rhs=ts_list[m],
                        start=(m == 0),
                        stop=(m == SM - 1),
                    )
                # exp row chunk (no max subtraction; scores are O(1)) and
                # the chunk's softmax partial sum
                st = state[b]
                nc.scalar.activation(
                    out=st["exp_row"][:, c * schunk : (c + 1) * schunk],
                    in_=psc,
                    func=AF.Exp,
                    accum_out=st["tparts"][:, c : c + 1],
                )
                if c == NCH - 1:
                    # softmax denominator + normalized weights can go out now;
                    # the raw exp_row stays untouched for the deferred context
                    tsum = rows2.tile([1, 1], fp32, tag="tsum", name=f"tsum_{b}")
                    nc.vector.reduce_sum(
                        out=tsum, in_=st["tparts"], axis=mybir.AxisListType.X
                    )
                    invt = rows2.tile([1, 1], fp32, tag="invt", name=f"invt_{b}")
                    nc.vector.reciprocal(out=invt, in_=tsum)
                    st["invt"] = invt
                    w_norm = rows.tile([1, s], fp32, tag="w_norm", name=f"wn_{b}")
                    nc.vector.tensor_scalar_mul(
                        out=w_norm, in0=st["exp_row"], scalar1=invt
                    )
                    wq_ = nc.sync if b == bpc - 1 else nc.gpsimd
                    wq_.dma_start(out=w_out[b : b + 1, :], in_=w_norm)
                    st["ctx_row"] = rows.tile(
                        [1, d2], fp32, tag="ctx_row", name=f"cr_{b}"
                    )

            emit_finish(len(seq) - 1)

    nc.compile()
    return nc


def _prep_core_inputs(q_last, keys_bf, b0, bpc, s, h, d2, schunk):
    """Host-side layout prep for one core: slice this core's batches and
    swizzle into the exact DRAM layouts the kernel DMAs from. Layout/dtype
    only -- no arithmetic."""
    import ml_dtypes

    bf16 = ml_dtypes.bfloat16
    SD = d2 // 128
    SJ = h // 128
    NCH = s // schunk
    SPC = schunk // 128

    kn = np.empty((bpc * NCH, 128, SPC, d2), dtype=bf16)
    kt = np.empty((bpc * NCH, 128, SD, schunk), dtype=bf16)
    for b in range(bpc):
        ks = keys_bf[:, b0 + b, :]  # [s, d2] (strided view)
        # kn[b,c][p, i, x] = ks[c*schunk + i*128 + p, x]
        kn[b * NCH : (b + 1) * NCH] = ks.reshape(NCH, SPC, 128, d2).transpose(
            0, 2, 1, 3
        )
        # kt[b,c][p, dd, x] = ks[c*schunk + x, dd*128 + p]
        kt[b * NCH : (b + 1) * NCH] = ks.reshape(NCH, schunk, SD, 128).transpose(
            0, 3, 2, 1
        )

    # qt[p, j, b] = q_last[b0+b, j*128+p]
    qt = np.ascontiguousarray(
        q_last[b0 : b0 + bpc].T.reshape(SJ, 128, bpc).transpose(1, 0, 2)
    ).astype(bf16)
    return {"qt": qt, "kn": kn, "kt": kt}


def _make_in_maps(inputs):
    import ml_dtypes

    bf16 = ml_dtypes.bfloat16
    q_last = np.ascontiguousarray(
        np.asarray(inputs["query"], dtype=np.float32)[:, -1, :]
    )  # [B, H]
    keys = np.asarray(inputs["keys"], dtype=np.float32)  # [S, B, 2H]
    keys_bf = keys.astype(bf16)
    wa = np.asarray(inputs["Wa_w"], dtype=np.float32)  # [H, H]
    ua = np.asarray(inputs["Ua_w"], dtype=np.float32)  # [H, 2H]
    va = np.asarray(inputs["Va_w"], dtype=np.float32).reshape(1, H)
    wab = np.asarray(inputs["Wa_b"], dtype=np.float32).reshape(H)
    uab = np.asarray(inputs["Ua_b"], dtype=np.float32).reshape(H)

    SD = D2 // 128
    SJ = H // 128
    SM = H // 128
    # uat[p, dd, j] = Ua_w[j, dd*128+p]
    uat = np.ascontiguousarray(
        ua.T.reshape(SD, 128, H).transpose(1, 0, 2)
    ).astype(bf16)
    # wat[p, jj, ho] = Wa_w[ho, jj*128+p]
    wat = np.ascontiguousarray(
        wa.T.reshape(SJ, 128, H).transpose(1, 0, 2)
    ).astype(bf16)
    # vac[p, m] = Va_w[0, m*128+p]
    vac = np.ascontiguousarray(va.reshape(SM, 128).T).astype(bf16)
    wabc = np.ascontiguousarray(wab.reshape(SM, 128).T)
    uabc = np.ascontiguousarray(uab.reshape(SM, 128).T)

    in_maps = []
    for c in range(NCORES):
        m = _prep_core_inputs(q_last, keys_bf, c * BPC, BPC, S, H, D2, 512)
        m.update(
            {"uat": uat, "wat": wat, "vac": vac, "wabc": wabc, "uabc": uabc}
        )
        in_maps.append(m)
    return in_maps


def run(inputs, trace=False, **kwargs):
    """Run on all 8 cores; returns ((context, weights), BassKernelResults)."""
    from concourse.bass_utils import run_bass_kernel_spmd

    if "nc" not in _CACHE:
        _CACHE["nc"] = _build()
    nc = _CACHE["nc"]
    in_maps = _make_in_maps(inputs)
    res = run_bass_kernel_spmd(
        nc, in_maps, core_ids=list(range(NCORES)), trace=trace, **kwargs
    )
    context = np.empty((B, 1, D2), dtype=np.float32)
    weights = np.empty((B, 1, S), dtype=np.float32)
    for c in range(NCORES):
        b0 = c * BPC
        context[b0 : b0 + BPC, 0, :] = res.results[c]["ctx"]
        weights[b0 : b0 + BPC, 0, :] = res.results[c]["wts"]
    return (context, weights), res


def kernel(**inputs):
    out, _ = run(inputs)
    return out
